# revision 3
# baseline (speedup 1.0000x reference)
"""Locally-connected convolution (unshared weights) on 8 Trainium2 NeuronCores.

out[b,o,i,j] = sum_{c,u,v} x[b,c,i+u,j+v] * weight[i,j,o,c,u,v]
  x: [64, 64, 32, 32] f32, weight: [28, 28, 128, 64, 5, 5] f32 -> out [64, 128, 28, 28]

Strategy ("flipped pairs" layout): each of the 784 output positions is an
independent GEMM [K=1600] x [K=1600, O=128] over B=64 batch vectors.  Shard
the 784 positions across 8 cores (98 each, raster-contiguous).

Weights are cast host-side to float8 E3M4 (x32 scale; x is pre-scaled by
1/32 in fp16).  The weight stream (20.07 MB/core, used exactly once) is the
hard floor: HBM->SBUF at ~350 GB/s takes ~60 us, so the tensor engine must
stay OFF the critical path.  The baseline made weights the PE-stationary
operand, paying a fresh LDWEIGHTS per matmul (zero weight reuse) which left
PE marginally too slow and HAM-throttled.  This version FLIPS the matmul:

  x is stationary [K, M=64 batch], weights are the moving rhs [K, N=O=128],
  out = [batch, O] in PSUM.  The two positions of an (even, odd) column pair
  occupy the two column halves of the PE array (tile_position (0,0) and
  (0,64)), so their weight streams run CONCURRENTLY through disjoint array
  halves: weight ingestion hits 2 cols/cycle @ 2.4 GHz, ~2.3x the baseline's
  LDWEIGHTS-path rate.  The stationary x of both positions is one contiguous
  [128, 128] SBUF slice, loaded by a single explicit LDWEIGHTS (FWL-eligible);
  the pair's matmuls are emitted with ldweights=False so they reuse the
  loaded array.  A nosync dependency chain pins tensor-queue program order
  (the PE-array state is a resource the Tile tracker doesn't model).

K decomposition per position (1600 = 25 taps x 64 ch) is unchanged:
  x stored once per core in "column pair" layout: partitions 0-63 hold
  channels of EVEN input columns, 64-127 of ODD columns; a [128, 64] slice
  at (row h, colpair cp) is one position's K=128 stationary chunk.  10 pair
  chunks + 5 leftover K=64 lone taps per position; the lone taps of the two
  parities run as concurrent 64x64 quadrant matmuls ((0,0) and (64,64)).

Schedule: weight DMAs carry two position-pairs each (6400 B/partition) on
the two HWDGE queues (16 SDMA engines round-robin: both queues busy =>
~350 GB/s); a 16-deep tile pool decouples the stream from PE progress.
PSUM->SBUF leaves as one [128, O] cast per pair; outputs stream in
7-pair fp16 stripes via gpsimd.  Output layout: [g*64+b, pair, o] for
position 2*pair+g.
"""

import numpy as np

B, C, H, W = 64, 64, 32, 32
ROWS = COLS = 28
O, KH, KW = 128, 5, 5
NCORES = 8
PPC = (ROWS * COLS) // NCORES  # 98 positions per core
NPAIR = PPC // 2               # 49 (even, odd) position pairs
XROWS, XW = 8, 36              # sheared x grid: 8 input rows x 36 cols
PAIRS = XW // 2                # 18 column pairs per sheared row
PAIRB = 2 * 10 * O + 5 * O     # weight bytes per partition per pair = 3200
WSCALE = 32.0                  # weights x32 into E3M4 range; x carries /32
OBLKP = 7                      # pairs per output block/DMA
WLA = 16                       # wtile pool depth (weight groups in flight)
XHALF = 9 * B                  # x row loaded in two 9-pair halves


def _core_geom(k):
    p0 = PPC * k
    return p0 // COLS, p0 % COLS  # r0 (first input/output row), s0 in {0, 14}


def _pos_slot(t):
    """Relative position t in [0,98) -> (di, w2) grid coords shared by all cores."""
    di, jj = t // COLS, t % COLS
    return di, jj + (4 if jj >= 14 else 0)


def _pair_geom(p):
    """Pair p -> (di, c): positions 2p (w2=2c) and 2p+1 (w2=2c+1), same di."""
    di, w2 = _pos_slot(2 * p)
    return di, w2 // 2


def _build_xs(x_chwb, k):
    """x_chwb: [C,H,W,B] f32 -> sheared per-core grid [C, XROWS, XW, B]."""
    r0, s0 = _core_geom(k)
    xs = np.zeros((C, XROWS, XW, B), dtype=x_chwb.dtype)
    for h in range(XROWS):
        if s0 == 0:
            xs[:, h, 0:18] = x_chwb[:, r0 + h, 0:18]
            xs[:, h, 18:36] = x_chwb[:, r0 + h, 14:32]
        else:
            xs[:, h, 0:18] = x_chwb[:, r0 + h, 14:32]
            if r0 + h + 1 < H:
                xs[:, h, 18:36] = x_chwb[:, r0 + h + 1, 0:18]
    return xs


def _build_xp(x_chwb, k):
    """-> [XROWS, 128, PAIRS*B] f16, partition g*64+c = channel c of col 2cp+g."""
    xs = _build_xs(x_chwb, k) * np.float32(1.0 / WSCALE)
    # [C, XROWS, PAIRS, 2, B] -> [2, C, XROWS, PAIRS, B]
    xg = xs.reshape(C, XROWS, PAIRS, 2, B).transpose(3, 0, 1, 2, 4)
    xp = xg.reshape(128, XROWS, PAIRS * B).transpose(1, 0, 2)
    return np.ascontiguousarray(xp).astype(np.float16)


def _abs_pos(k, t):
    p = PPC * k + t
    return p // COLS, p % COLS


def _build_wt(weight, k):
    """weight [ROWS,COLS,O,C,KH,KW] f32 -> per-core [128, NPAIR*PAIRB] E3M4.

    Per pair p (positions te=2p, to=2p+1), per-partition byte layout:
      [0,1280):    te pair chunks, (u,q)-major, each [128, O]; partition
                   g*64+c holds w[o, c, u, v = 2q + g + par(t)]
      [1280,1920): 5 shared lone blocks [128, O]: partitions 0:64 = te tap
                   (u, 4), 64:128 = to tap (u, 0)
      [1920,3200): to pair chunks
    (te's operands live in [0,1920) so the leading DMA of pair 0 can be
    split there for a faster pipeline start.)
    """
    import ml_dtypes

    ii, jj = zip(*[_abs_pos(k, t) for t in range(PPC)])
    wc = weight[list(ii), list(jj)]  # [PPC, O, C, KH, KW] f32
    # t parity == jj parity == w2 parity for every core (offsets are even)
    WT = np.zeros((2, C, NPAIR, PAIRB), np.float32)  # [g, c, p, col]
    for u in range(KH):
        for q in range(2):
            for g in range(2):
                for half in range(2):
                    v = 2 * q + g + half
                    blk = wc[half::2, :, :, u, v]  # [NPAIR, O, C]
                    col = half * 1920 + (2 * u + q) * O
                    WT[g, :, :, col:col + O] = blk.transpose(2, 0, 1)
        WT[0, :, :, 1280 + u * O:1280 + (u + 1) * O] = wc[0::2, :, :, u, 4].transpose(2, 0, 1)
        WT[1, :, :, 1280 + u * O:1280 + (u + 1) * O] = wc[1::2, :, :, u, 0].transpose(2, 0, 1)
    wt = np.ascontiguousarray(WT.reshape(128, NPAIR * PAIRB) * np.float32(WSCALE))
    return wt.astype(ml_dtypes.float8_e3m4)


def _emulate_core(xp, wt):
    """Pure-numpy emulation of the device program (mirrors AP arithmetic)."""
    xpf = xp.astype(np.float32)                      # [8, 128, PAIRS*B]
    wtf = wt.astype(np.float32).reshape(128, NPAIR, PAIRB)
    out = np.zeros((128, NPAIR, O), np.float32)      # [g*64+b, pair, o]
    for p in range(NPAIR):
        di, c = _pair_geom(p)
        accA = np.zeros((B, O), np.float32)
        accB = np.zeros((B, O), np.float32)
        for u in range(KH):
            h = di + u
            for q in range(2):
                accA += xpf[h][:, (c + q) * B:(c + q + 1) * B].T @ \
                    wtf[:, p, (2 * u + q) * O:(2 * u + q + 1) * O]
                accB += xpf[h][:, (c + q + 1) * B:(c + q + 2) * B].T @ \
                    wtf[:, p, 1920 + (2 * u + q) * O:1920 + (2 * u + q + 1) * O]
            accA += xpf[h][0:64, (c + 2) * B:(c + 3) * B].T @ \
                wtf[0:64, p, 1280 + u * O:1280 + (u + 1) * O]
            accB += xpf[h][64:128, c * B:(c + 1) * B].T @ \
                wtf[64:128, p, 1280 + u * O:1280 + (u + 1) * O]
        out[0:64, p, :] = accA
        out[64:128, p, :] = accB
    return out  # scale already folded via x/32 * w*32


def _assemble(outs):
    """list of 8 per-core [128, NPAIR*O] (any shape, flat order) -> [B,O,ROWS,COLS]."""
    percore = []
    for o in outs:
        a = np.asarray(o, np.float32).reshape(2, B, NPAIR, O)
        percore.append(a.transpose(2, 0, 1, 3).reshape(PPC, B, O))
    full = np.concatenate(percore, axis=0)           # [784, B, O]
    return np.ascontiguousarray(full.transpose(1, 2, 0)).reshape(B, O, ROWS, COLS)


_PROG_CACHE = {}


def _build_program():
    if "nc" in _PROG_CACHE:
        return _PROG_CACHE["nc"]
    import concourse.bass as bass
    import concourse.tile as tile
    from concourse import bacc, mybir

    f8, f16, f32 = mybir.dt.float8e3, mybir.dt.float16, mybir.dt.float32
    NOSYNC = mybir.DependencyInfo.NO_SYNC_ONLY
    nc = bacc.Bacc("TRN2", target_bir_lowering=False, debug=False, num_devices=NCORES)
    xp_d = nc.dram_tensor("xp", [XROWS, 128, PAIRS * B], f16, kind="ExternalInput")
    wt_d = nc.dram_tensor("wt", [128, NPAIR * PAIRB], f8, kind="ExternalInput")
    out_d = nc.dram_tensor("out", [128, NPAIR * O], f16, kind="ExternalOutput")

    with tile.TileContext(nc) as tc:
        with tc.tile_pool(name="xpool", bufs=1) as xpool, \
             tc.tile_pool(name="wpool", bufs=WLA) as wpool, \
             tc.tile_pool(name="opool", bufs=3) as opool, \
             tc.tile_pool(name="psum", bufs=8, space="PSUM") as ppool:
            xp, wt, outp = xp_d.ap(), wt_d.ap(), out_d.ap()
            XT = [xpool.tile([128, PAIRS * B], f16, name=f"x{h}", tag=f"x{h}")
                  for h in range(XROWS)]
            # Weight DMAs carry GROUPS[i] pairs each: big transfers amortize
            # the per-DMA SEQ/DGE overhead so the two HWDGE queues keep the
            # 16 DMA engines saturated; the leading groups stay small (and
            # pair 0 is split) for a fast pipeline start.
            GROUPS = [1, 1] + [2] * 23 + [1]
            g0 = [0]
            for n in GROUPS:
                g0.append(g0[-1] + n)
            pair_loc = {}
            for gi, n in enumerate(GROUPS):
                for l in range(n):
                    pair_loc[g0[gi] + l] = (gi, l * PAIRB)
            wtiles = [wpool.tile([128, n * PAIRB], f8, name=f"w{gi}", tag="wt")
                      for gi, n in enumerate(GROUPS)]
            weng = [nc.scalar, nc.sync]

            def load_x(h, hf, eng):
                eng.dma_start(XT[h][:, hf * XHALF:(hf + 1) * XHALF],
                              xp[h, :, hf * XHALF:(hf + 1) * XHALF])

            def load_w(gi):
                c0, c1 = g0[gi] * PAIRB, g0[gi + 1] * PAIRB
                weng[gi % 2].dma_start(wtiles[gi][:], wt[:, c0:c1])

            # Emission order == per-queue FIFO order.  Pair 0 needs x rows
            # 0-4 (first halves) + its weight slice; x rows 5-7 only matter
            # from p=14, second halves (xb) from p=7.
            nc.sync.dma_start(wtiles[0][:, 0:1920], wt[:, 0:1920])
            for h in range(3):
                load_x(h, 0, nc.scalar)
            nc.sync.dma_start(wtiles[0][:, 1920:PAIRB], wt[:, 1920:PAIRB])
            load_x(3, 0, nc.scalar)
            load_x(4, 0, nc.scalar)
            load_w(1)   # sync
            load_w(2)   # scalar
            load_w(3)   # sync
            for h in range(5, XROWS):
                load_x(h, 0, nc.scalar)
            load_w(4)
            for h in range(0, XROWS, 2):
                load_x(h, 1, nc.sync)
            for h in range(1, XROWS, 2):
                load_x(h, 1, nc.scalar)
            for gi in range(5, len(GROUPS)):
                load_w(gi)  # flow-controlled by wpool depth

            # Tensor-queue program order is load-bearing (explicit LDWEIGHTS
            # + non-self-loading matmuls share PE-array state): chain every
            # tensor instruction with a nosync dep so the Tile scheduler
            # cannot reorder within the queue.
            tprev = [None]

            def chain(bi):
                if tprev[0] is not None:
                    bi.ins.add_dependency(tprev[0], NOSYNC)
                tprev[0] = bi.ins.name
                return bi

            def emit_pair_section(p, ps):
                di, c = _pair_geom(p)
                gi, poff = pair_loc[p]
                wti = wtiles[gi]
                for u in range(KH):
                    h = di + u
                    for q in range(2):
                        start = (u == 0 and q == 0)
                        chain(nc.tensor.ldweights(
                            XT[h][:, (c + q) * B:(c + q + 2) * B]))
                        woff = poff + (2 * u + q) * O
                        mmA = nc.tensor.matmul(
                            ps[0:64, :],
                            XT[h][:, (c + q) * B:(c + q + 1) * B],
                            wti[:, woff:woff + O],
                            start=start, stop=False)
                        mmA.ins.ldweights = False
                        chain(mmA)
                        mmB = nc.tensor.matmul(
                            ps[64:128, :],
                            XT[h][:, (c + q + 1) * B:(c + q + 2) * B],
                            wti[:, 1920 + woff:1920 + woff + O],
                            start=start, stop=False)
                        mmB.ins.ldweights = False
                        chain(mmB)

            def emit_lone_section(p, ps):
                di, c = _pair_geom(p)
                gi, poff = pair_loc[p]
                wti = wtiles[gi]
                for u in range(KH):
                    h = di + u
                    stop = (u == KH - 1)
                    woff = poff + 1280 + u * O
                    chain(nc.tensor.matmul(
                        ps[0:64, :],
                        XT[h][0:64, (c + 2) * B:(c + 3) * B],
                        wti[0:64, woff:woff + O],
                        start=False, stop=stop))
                    chain(nc.tensor.matmul(
                        ps[64:128, :],
                        XT[h][64:128, c * B:(c + 1) * B],
                        wti[64:128, woff:woff + O],
                        start=False, stop=stop))

            for p0 in range(0, NPAIR, OBLKP):
                otile = opool.tile([128, OBLKP * O], f16, tag="ot")
                pss = {}
                for p in range(p0, p0 + OBLKP):
                    pss[p] = ppool.tile([128, O], f32, name="ps", tag="ps")
                    emit_pair_section(p, pss[p])
                for p in range(p0, p0 + OBLKP):
                    emit_lone_section(p, pss[p])
                    nc.vector.tensor_copy(
                        otile[:, (p - p0) * O:(p - p0 + 1) * O], pss[p][:])
                oeng = nc.sync if p0 + OBLKP >= NPAIR else nc.gpsimd
                oeng.dma_start(outp[:, p0 * O:(p0 + OBLKP) * O], otile[:])

    nc.compile()
    _PROG_CACHE["nc"] = nc
    return nc


def _make_in_maps(x, weight):
    x_chwb = np.ascontiguousarray(
        np.asarray(x, np.float32).transpose(1, 2, 3, 0))
    w32 = np.asarray(weight, np.float32)
    return [{"xp": _build_xp(x_chwb, k), "wt": _build_wt(w32, k)}
            for k in range(NCORES)]


def kernel(x, weight):
    from concourse.bass_utils import run_bass_kernel_spmd

    nc = _build_program()
    in_maps = _make_in_maps(x, weight)
    res = run_bass_kernel_spmd(nc, in_maps, core_ids=list(range(NCORES)))
    return _assemble([res.results[k]["out"] for k in range(NCORES)])


# revision 5
# speedup vs baseline: 2.3022x; 2.3022x over previous
"""Locally-connected convolution (unshared weights) on 8 Trainium2 NeuronCores.

out[b,o,i,j] = sum_{c,u,v} x[b,c,i+u,j+v] * weight[i,j,o,c,u,v]
  x: [64, 64, 32, 32] f32, weight: [28, 28, 128, 64, 5, 5] f32 -> out [64, 128, 28, 28]

Strategy ("pairs" layout): each of the 784 output positions is an
independent GEMM [K=1600] x [K=1600, O=128] over B=64 batch vectors.  Shard
the 784 positions across 8 cores (98 each, raster-contiguous).

Weights are cast host-side to float8 E3M4 (x32 scale; x is pre-scaled by
1/32 in fp16 so no on-device rescale is needed).  This halves the dominant
HBM traffic (weights are used exactly once; relerr ~1.2e-2 vs the 2e-2
gate).  The matmul is "flipped": weights are the stationary lhsT
[K, O=128], x the moving rhs [K, B=64] (mixed fp8e3 x fp16 operands), so
each matmul streams 64 rows and fills all 128 PSUM partitions.

K decomposition per position (1600 = 25 taps x 64 ch):
  x is stored ONCE per core in "column pair" layout: partitions 0-63 hold
  channels of EVEN input columns, partitions 64-127 channels of ODD columns;
  free axis is (row h, column-pair cp, batch b).  A [128, 64] slice at
  (h, cp) yields two adjacent-column taps at K=128.  Positions alternate
  column parity, so every position gets 10 such pair chunks (u x 2) plus 5
  leftover single taps (K=64, lower partitions for even positions / upper
  for odd).  Leftover-tap weights of an (even, odd) position pair share one
  [128, 128] block -> zero padding in the weight stream (20.07 MB/core).

Schedule: weight DMAs carry two position-pairs each (6400 B/partition) on
the two HWDGE queues so the 16 DMA engines stay saturated (~420 GB/s
observed); a 16-deep tile pool decouples the stream from PE progress.  Per
7-position output block, all seventy K=128 matmuls are emitted first, then
the K=64 leftovers grouped by parity (PE 128<->64 tile-config switches per
block instead of per position); PSUM->SBUF casts chase each position's
last accumulate and outputs leave in 7-position fp16 stripes via gpsimd.
Measured ~93.6 us on 8 axon trn2 cores (baseline fp16 kernel: 189.5 us).
"""

import numpy as np

B, C, H, W = 64, 64, 32, 32
ROWS = COLS = 28
O, KH, KW = 128, 5, 5
NCORES = 8
PPC = (ROWS * COLS) // NCORES  # 98 positions per core
NPAIR = PPC // 2               # 49 (even, odd) position pairs
XROWS, XW = 8, 36              # sheared x grid: 8 input rows x 36 cols
PAIRS = XW // 2                # 18 column pairs per sheared row
PAIRB = 2 * 10 * O + 5 * O     # weight bytes per partition per pair = 3200
WSCALE = 32.0                  # weights x32 into E3M4 range; x carries /32
OBLK = 7                       # positions per output block/DMA
WLA = 25                       # wtile pool depth (whole stream resident)
XHALF = 9 * B                  # x row loaded in two 9-pair halves
XRROWS = 6                     # row-pair workspace rows (di+u0 <= 5)


def _core_geom(k):
    p0 = PPC * k
    return p0 // COLS, p0 % COLS  # r0 (first input/output row), s0 in {0, 14}


def _pos_slot(t):
    """Relative position t in [0,98) -> (di, w2) grid coords shared by all cores."""
    di, jj = t // COLS, t % COLS
    return di, jj + (4 if jj >= 14 else 0)


def _build_xs(x_chwb, k):
    """x_chwb: [C,H,W,B] f32 -> sheared per-core grid [C, XROWS, XW, B]."""
    r0, s0 = _core_geom(k)
    xs = np.zeros((C, XROWS, XW, B), dtype=x_chwb.dtype)
    for h in range(XROWS):
        if s0 == 0:
            xs[:, h, 0:18] = x_chwb[:, r0 + h, 0:18]
            xs[:, h, 18:36] = x_chwb[:, r0 + h, 14:32]
        else:
            xs[:, h, 0:18] = x_chwb[:, r0 + h, 14:32]
            if r0 + h + 1 < H:
                xs[:, h, 18:36] = x_chwb[:, r0 + h + 1, 0:18]
    return xs


def _build_xp(x_chwb, k):
    """-> [XROWS, 128, PAIRS*B] f16, partition g*64+c = channel c of col 2cp+g."""
    xs = _build_xs(x_chwb, k) * np.float32(1.0 / WSCALE)
    # [C, XROWS, PAIRS, 2, B] -> [2, C, XROWS, PAIRS, B]
    xg = xs.reshape(C, XROWS, PAIRS, 2, B).transpose(3, 0, 1, 2, 4)
    xp = xg.reshape(128, XROWS, PAIRS * B).transpose(1, 0, 2)
    return np.ascontiguousarray(xp).astype(np.float16)


def _abs_pos(k, t):
    p = PPC * k + t
    return p // COLS, p % COLS


def _build_wt(weight, k):
    """weight [ROWS,COLS,O,C,KH,KW] f32 -> per-core [128, NPAIR*PAIRB] E3M4.

    Per pair p (positions te=2p, to=2p+1), per-partition byte layout:
      [0,1280):    te pair chunks, (u,q)-major, each [128, O]; partition
                   g*64+c holds w[o, c, u, v = 2q + g + par(t)]
      [1280,1920): 5 shared lone blocks [128, O]: partitions 0:64 = te tap
                   (u, 4), 64:128 = to tap (u, 0)
      [1920,3200): to pair chunks
    (te's operands live in [0,1920) so the leading DMA of pair 0 can be
    split there for a faster pipeline start.)
    """
    import ml_dtypes

    ii, jj = zip(*[_abs_pos(k, t) for t in range(PPC)])
    wc = weight[list(ii), list(jj)]  # [PPC, O, C, KH, KW] f32
    # t parity == jj parity == w2 parity for every core (offsets are even)
    WT = np.zeros((2, C, NPAIR, PAIRB), np.float32)  # [g, c, p, col]
    for u in range(KH):
        for q in range(2):
            for g in range(2):
                for half in range(2):
                    v = 2 * q + g + half
                    blk = wc[half::2, :, :, u, v]  # [NPAIR, O, C]
                    col = half * 1920 + (2 * u + q) * O
                    WT[g, :, :, col:col + O] = blk.transpose(2, 0, 1)
        WT[0, :, :, 1280 + u * O:1280 + (u + 1) * O] = wc[0::2, :, :, u, 4].transpose(2, 0, 1)
        WT[1, :, :, 1280 + u * O:1280 + (u + 1) * O] = wc[1::2, :, :, u, 0].transpose(2, 0, 1)
    wt = np.ascontiguousarray(WT.reshape(128, NPAIR * PAIRB) * np.float32(WSCALE))
    return wt.astype(ml_dtypes.float8_e3m4)


def _chunks(t):
    """Position t -> (di, 10 pair descriptors, 5 lone descriptors).

    Descriptors are (kind, u, cp, woff, g).  The device emits all pair
    chunks of an output block, then the lone (K=64) chunks grouped by
    parity, so 128<->64 PE tile-config switches happen per block, not per
    position.
    """
    di, w2 = _pos_slot(t)
    par = w2 % 2
    half = t % 2
    cp0 = (w2 + par) // 2
    cpl = (w2 + 4) // 2 if par == 0 else (w2 - 1) // 2
    pairs = [("pair", u, cp0 + q, half * 1920 + (2 * u + q) * O, 0)
             for u in range(KH) for q in range(2)]
    lones = [("lone", u, cpl, 1280 + u * O, par) for u in range(KH)]
    return di, pairs, lones


def _emulate_core(xp, wt, xr=None):
    """Pure-numpy emulation of the device program (mirrors AP arithmetic)."""
    xpf = xp.astype(np.float32)                      # [8, 128, PAIRS*B]
    wtf = wt.astype(np.float32).reshape(128, NPAIR, PAIRB)
    out = np.zeros((128, PPC, B), np.float32)
    for t in range(PPC):
        p = t // 2
        di, pairs, lones = _chunks(t)
        acc = np.zeros((128, B), np.float32)
        for kind, u, cp, woff, g in pairs + lones:
            if kind == "pair":
                lhsT = wtf[:, p, woff:woff + O]          # [128, 128]
                rhs = xpf[di + u, :, cp * B:(cp + 1) * B]  # [128, 64]
            else:
                lhsT = wtf[g * 64:(g + 1) * 64, p, woff:woff + O]
                rhs = xpf[di + u, g * 64:(g + 1) * 64, cp * B:(cp + 1) * B]
            acc += lhsT.T @ rhs
        out[:, t, :] = acc
    return out  # [O, PPC, B]; scale already folded via x/32 * w*32


def _assemble(outs):
    """list of 8 per-core [128, PPC*B] -> [B, O, ROWS, COLS] f32."""
    full = np.concatenate(
        [np.asarray(o, np.float32).reshape(O, PPC, B) for o in outs], axis=1)
    return np.ascontiguousarray(full.transpose(2, 0, 1)).reshape(B, O, ROWS, COLS)


_PROG_CACHE = {}


def _build_program():
    if "nc" in _PROG_CACHE:
        return _PROG_CACHE["nc"]
    import concourse.bass as bass
    import concourse.tile as tile
    from concourse import bacc, mybir

    f8, f16, f32 = mybir.dt.float8e3, mybir.dt.float16, mybir.dt.float32
    nc = bacc.Bacc("TRN2", target_bir_lowering=False, debug=False, num_devices=NCORES)
    xp_d = nc.dram_tensor("xp", [XROWS, 128, PAIRS * B], f16, kind="ExternalInput")
    wt_d = nc.dram_tensor("wt", [128, NPAIR * PAIRB], f8, kind="ExternalInput")
    out_d = nc.dram_tensor("out", [128, PPC * B], f16, kind="ExternalOutput")

    with tile.TileContext(nc) as tc:
        with tc.tile_pool(name="xpool", bufs=1) as xpool, \
             tc.tile_pool(name="wpool", bufs=WLA) as wpool, \
             tc.tile_pool(name="opool", bufs=3) as opool, \
             tc.tile_pool(name="psum", bufs=8, space="PSUM") as ppool:
            xp, wt, outp = xp_d.ap(), wt_d.ap(), out_d.ap()
            XT = [xpool.tile([128, PAIRS * B], f16, name=f"x{h}", tag=f"x{h}")
                  for h in range(XROWS)]
            # Weight DMAs carry GROUPS[i] pairs each: big transfers amortize
            # the per-DMA SEQ/DGE overhead (~1.2us) so two HWDGE queues keep
            # the 16 DMA engines saturated; the leading groups stay small
            # (and pair 0 is split) for a fast pipeline start.
            GROUPS = [1, 1] + [2] * 23 + [1]
            g0 = [0]
            for n in GROUPS:
                g0.append(g0[-1] + n)
            pair_loc = {}
            for gi, n in enumerate(GROUPS):
                for l in range(n):
                    pair_loc[g0[gi] + l] = (gi, l * PAIRB)
            wtiles = [wpool.tile([128, n * PAIRB], f8, name=f"w{gi}", tag="wt")
                      for gi, n in enumerate(GROUPS)]
            weng = [nc.scalar, nc.sync]

            def load_x(h, hf, eng):
                eng.dma_start(XT[h][:, hf * XHALF:(hf + 1) * XHALF],
                              xp[h, :, hf * XHALF:(hf + 1) * XHALF])

            def load_w(gi):
                c0, c1 = g0[gi] * PAIRB, g0[gi + 1] * PAIRB
                weng[gi % 2].dma_start(wtiles[gi][:], wt[:, c0:c1])

            # Emission order == per-queue FIFO order.  Position 0 needs x
            # rows 0-4 (first halves) + pair 0's [0,1920) slice; x rows 5-7
            # only matter from t=28, second halves (xb) from t=14.
            nc.sync.dma_start(wtiles[0][:, 0:1920], wt[:, 0:1920])
            for h in range(3):
                load_x(h, 0, nc.scalar)
            nc.sync.dma_start(wtiles[0][:, 1920:PAIRB], wt[:, 1920:PAIRB])
            load_x(3, 0, nc.scalar)
            load_x(4, 0, nc.scalar)
            load_w(1)   # sync
            load_w(2)   # scalar
            load_w(3)   # sync
            for h in range(5, XROWS):
                load_x(h, 0, nc.scalar)
            load_w(4)
            for h in range(0, XROWS, 2):
                load_x(h, 1, nc.sync)
            for h in range(1, XROWS, 2):
                load_x(h, 1, nc.scalar)
            for gi in range(5, len(GROUPS)):
                load_w(gi)  # flow-controlled by wpool depth

            def mm(ps, p, di, ch, start, stop):
                kind, u, cp, woff, g = ch
                gi, poff = pair_loc[p]
                if kind == "pair":
                    lhsT = wtiles[gi][:, poff + woff:poff + woff + O]
                    rhs = XT[di + u][:, cp * B:(cp + 1) * B]
                else:
                    lhsT = wtiles[gi][g * 64:g * 64 + 64, poff + woff:poff + woff + O]
                    rhs = XT[di + u][g * 64:g * 64 + 64, cp * B:(cp + 1) * B]
                nc.tensor.matmul(ps[:], lhsT, rhs, start=start, stop=stop)

            for t0 in range(0, PPC, OBLK):
                otile = opool.tile([128, OBLK * B], f16, tag="ot")
                pss, parts = {}, {}
                for t in range(t0, t0 + OBLK):
                    di, pairs, lones = _chunks(t)
                    ps = ppool.tile([128, B], f32, tag="ps")
                    pss[t], parts[t] = ps, (di, lones)
                    for i, ch in enumerate(pairs):
                        mm(ps, t // 2, di, ch, start=(i == 0), stop=False)
                for par in (t0 % 2, 1 - t0 % 2):
                    for t in range(t0, t0 + OBLK):
                        if t % 2 != par:
                            continue
                        di, lones = parts[t]
                        for i, ch in enumerate(lones):
                            mm(pss[t], t // 2, di, ch, start=False, stop=(i == KH - 1))
                        nc.vector.tensor_copy(
                            otile[:, (t - t0) * B:(t - t0 + 1) * B], pss[t][:])
                oeng = nc.sync if t0 + OBLK >= PPC else nc.gpsimd
                oeng.dma_start(outp[:, t0 * B:(t0 + OBLK) * B], otile[:])

    nc.compile()
    _PROG_CACHE["nc"] = nc
    return nc


def _make_in_maps(x, weight):
    x_chwb = np.ascontiguousarray(
        np.asarray(x, np.float32).transpose(1, 2, 3, 0))
    w32 = np.asarray(weight, np.float32)
    return [{"xp": _build_xp(x_chwb, k), "wt": _build_wt(w32, k)}
            for k in range(NCORES)]


def kernel(x, weight):
    from concourse.bass_utils import run_bass_kernel_spmd

    nc = _build_program()
    in_maps = _make_in_maps(x, weight)
    res = run_bass_kernel_spmd(nc, in_maps, core_ids=list(range(NCORES)))
    return _assemble([res.results[k]["out"] for k in range(NCORES)])



# revision 7
# speedup vs baseline: 2.3696x; 1.0293x over previous
"""Locally-connected convolution (unshared weights) on 8 Trainium2 NeuronCores.

out[b,o,i,j] = sum_{c,u,v} x[b,c,i+u,j+v] * weight[i,j,o,c,u,v]
  x: [64, 64, 32, 32] f32, weight: [28, 28, 128, 64, 5, 5] f32 -> out [64, 128, 28, 28]

"x-chunk-major" flipped layout: the weight stream (20.07 MB/core fp8, used
exactly once) is the hard floor (~60 us at ~350 GB/s HBM->SBUF), so the PE
must ingest weights faster than DMA delivers them.  Weights are the MOVING
matmul operand (2 concurrent col-tiled streams = 2 cols/cycle @ 2.4 GHz);
x chunks are the stationary operand, and the weight stream is reordered so
one stationary load serves up to 256 moving columns:

  For a stripe of 4 (even, odd) position pairs sharing an output row, the
  PSUM accumulator is one bank [128, 4*O] f32: partitions 0-63 = batch of
  the 4 even positions, 64-127 = odd; free = (pair slot, o).  An x chunk
  [128, 64] at (row h, colpair cp) is consumed by two adjacent even
  positions' weight blocks -> ONE matmul, rhs [128, 256], out two adjacent
  PSUM slots.  LDWEIGHTS (64 cols) hides under the 256-col stream.  Even
  and odd positions ride the two column halves of the PE array
  (tile_position (0,0) / (0,64)) so their streams run concurrently.
  Accumulation relies on per-element has_written: only the first matmul
  per (stripe, half) uses start=True (whole-bank clear); later matmuls
  overwrite-on-first-touch / accumulate, which is order-independent.
  The 5 leftover K=64 lone taps per position run as 64x64 quadrant
  matmuls ((0,0) vs (64,64)), two parities concurrent, sharing [128, O]
  weight blocks.  A nosync dependency chain pins tensor-queue program
  order (PE-array state + bank clears are resources the Tile dependency
  tracker doesn't model).

K decomposition per position (1600 = 25 taps x 64 ch) and the x layout are
unchanged from the pairs kernel: x stored once per core, partitions 0-63 =
channels of EVEN input columns, 64-127 ODD; 10 pair chunks + 5 lone taps.

Schedule: weight DMAs carry ~6400 B/partition segments cut at matmul
boundaries, alternating the two HWDGE queues (16 SDMA engines round-robin:
both queues busy => ~350 GB/s); a deep tile pool keeps DMA independent of
PE progress.  One [128, 4*O] f32->f16 cast per stripe; outputs stream per
stripe via gpsimd.  Output layout: [g*64+b, pair, o] for position 2*pair+g.
"""

import numpy as np

B, C, H, W = 64, 64, 32, 32
ROWS = COLS = 28
O, KH, KW = 128, 5, 5
NCORES = 8
PPC = (ROWS * COLS) // NCORES  # 98 positions per core
NPAIR = PPC // 2               # 49 (even, odd) position pairs
XROWS, XW = 8, 36              # sheared x grid: 8 input rows x 36 cols
PAIRS = XW // 2                # 18 column pairs per sheared row
WSCALE = 32.0                  # weights x32 into E3M4 range; x carries /32
SP = 4                         # pairs per stripe (= one PSUM bank)
WLA = 20                       # wtile pool depth (weight segments in flight)
XHALF = 9 * B                  # x row loaded in two 9-pair halves
SEG = 6400                     # target weight-DMA segment size (B/partition)


def _core_geom(k):
    p0 = PPC * k
    return p0 // COLS, p0 % COLS  # r0 (first input/output row), s0 in {0, 14}


def _pos_slot(t):
    """Relative position t in [0,98) -> (di, w2) grid coords shared by all cores."""
    di, jj = t // COLS, t % COLS
    return di, jj + (4 if jj >= 14 else 0)


def _pair_geom(p):
    """Pair p -> (di, c): positions 2p (w2=2c) and 2p+1 (w2=2c+1), same di."""
    di, w2 = _pos_slot(2 * p)
    return di, w2 // 2


def _stripes():
    """Stripes never straddle output rows (rows are 14 or 7 pairs)."""
    out = []
    for r0 in range(0, NPAIR, 14):
        row = list(range(r0, min(r0 + 14, NPAIR)))
        sizes = [4, 4, 3, 3] if len(row) == 14 else [4, 3]
        i = 0
        for sz in sizes:
            out.append(row[i:i + sz])
            i += sz
    return out


def _stripe_plan(pairs):
    """-> (di, cs, mms, ncols).  mms in emission order; weight columns are
    assigned in the same order (the A/B lone matmuls share one block).

    mm: dict(kind, half, u, cp, slot0, nblk, woff, blocks)
      pair: lhsT = XT[di+u][:, cp*B:(cp+1)*B], rhs = wt[:, woff:woff+nblk*O],
            out = stile[half*64:+64, slot0*O:(slot0+nblk)*O]
      lone: all three operands on partitions [half*64, half*64+64)
    """
    di, _ = _pair_geom(pairs[0])
    cs = [_pair_geom(p)[1] for p in pairs]
    n = len(pairs)
    mms = []
    off = 0
    for u in range(KH):
        per_half = []
        for half in range(2):
            blocks = [(s, q) for s in range(n) for q in range(2)]
            groups, i = [], 0
            while i < len(blocks):
                s, q = blocks[i]
                if (q == 1 and i + 1 < len(blocks)
                        and cs[blocks[i + 1][0]] == cs[s] + 1):
                    groups.append([blocks[i], blocks[i + 1]])
                    i += 2
                else:
                    groups.append([blocks[i]])
                    i += 1
            per_half.append(groups)
        ga, gb = per_half
        for j in range(max(len(ga), len(gb))):
            for half, gl in ((0, ga), (1, gb)):
                if j < len(gl):
                    grp = gl[j]
                    s0, q0 = grp[0]
                    mms.append(dict(kind="pair", half=half, u=u,
                                    cp=cs[s0] + q0 + half, slot0=s0,
                                    nblk=len(grp), woff=off, blocks=grp))
                    off += len(grp) * O
    for u in range(KH):
        for s in range(n):
            for half in range(2):
                mms.append(dict(kind="lone", half=half, u=u,
                                cp=cs[s] + (2 if half == 0 else 0),
                                slot0=s, nblk=1, woff=off, blocks=[(s, None)]))
            off += O  # one [128, O] block shared by the A and B lone matmuls
    return di, cs, mms, off


def _stripe_bases():
    bases, tot = [], 0
    for pairs in _stripes():
        bases.append(tot)
        tot += _stripe_plan(pairs)[3]
    return bases, tot


def _build_xs(x_chwb, k):
    """x_chwb: [C,H,W,B] f32 -> sheared per-core grid [C, XROWS, XW, B]."""
    r0, s0 = _core_geom(k)
    xs = np.zeros((C, XROWS, XW, B), dtype=x_chwb.dtype)
    for h in range(XROWS):
        if s0 == 0:
            xs[:, h, 0:18] = x_chwb[:, r0 + h, 0:18]
            xs[:, h, 18:36] = x_chwb[:, r0 + h, 14:32]
        else:
            xs[:, h, 0:18] = x_chwb[:, r0 + h, 14:32]
            if r0 + h + 1 < H:
                xs[:, h, 18:36] = x_chwb[:, r0 + h + 1, 0:18]
    return xs


def _build_xp(x_chwb, k):
    """-> [XROWS, 128, PAIRS*B] f16, partition g*64+c = channel c of col 2cp+g."""
    xs = _build_xs(x_chwb, k) * np.float32(1.0 / WSCALE)
    xg = xs.reshape(C, XROWS, PAIRS, 2, B).transpose(3, 0, 1, 2, 4)
    xp = xg.reshape(128, XROWS, PAIRS * B).transpose(1, 0, 2)
    return np.ascontiguousarray(xp).astype(np.float16)


def _abs_pos(k, t):
    p = PPC * k + t
    return p // COLS, p % COLS


def _build_wt(weight, k):
    """weight [ROWS,COLS,O,C,KH,KW] f32 -> per-core [128, WTOT] E3M4 in
    x-chunk-major stream order (exactly the _stripe_plan emission order)."""
    import ml_dtypes

    ii, jj = zip(*[_abs_pos(k, t) for t in range(PPC)])
    wc = weight[list(ii), list(jj)]  # [PPC, O, C, KH, KW] f32
    bases, tot = _stripe_bases()
    WT = np.zeros((128, tot), np.float32)
    for si, pairs in enumerate(_stripes()):
        di, cs, mms, ncols = _stripe_plan(pairs)
        base = bases[si]
        for m in mms:
            u = m["u"]
            if m["kind"] == "pair":
                t0 = m["half"]
                for bi, (s, q) in enumerate(m["blocks"]):
                    t = 2 * pairs[s] + t0
                    col = base + m["woff"] + bi * O
                    for g in range(2):
                        v = 2 * q + g + t0
                        # block[g*64+cch, o] = w[t][o, cch, u, v]
                        WT[g * 64:(g + 1) * 64, col:col + O] = \
                            wc[t, :, :, u, v].T
            elif m["half"] == 0:  # fill the shared lone block once
                s = m["slot0"]
                col = base + m["woff"]
                WT[0:64, col:col + O] = wc[2 * pairs[s], :, :, u, 4].T
                WT[64:128, col:col + O] = wc[2 * pairs[s] + 1, :, :, u, 0].T
    wt = np.ascontiguousarray(WT * np.float32(WSCALE))
    return wt.astype(ml_dtypes.float8_e3m4)


def _emulate_core(xp, wt):
    """Pure-numpy emulation of the device program (mirrors AP arithmetic)."""
    xpf = xp.astype(np.float32)
    wtf = wt.astype(np.float32)
    bases, _ = _stripe_bases()
    out = np.zeros((128, NPAIR, O), np.float32)
    for si, pairs in enumerate(_stripes()):
        di, cs, mms, ncols = _stripe_plan(pairs)
        base = bases[si]
        acc = np.zeros((128, len(pairs) * O), np.float32)
        for m in mms:
            h = di + m["u"]
            w0 = base + m["woff"]
            pl = m["half"] * 64
            if m["kind"] == "pair":
                lhsT = xpf[h][:, m["cp"] * B:(m["cp"] + 1) * B]
                rhs = wtf[:, w0:w0 + m["nblk"] * O]
                acc[pl:pl + 64, m["slot0"] * O:(m["slot0"] + m["nblk"]) * O] += \
                    lhsT.T @ rhs
            else:
                lhsT = xpf[h][pl:pl + 64, m["cp"] * B:(m["cp"] + 1) * B]
                rhs = wtf[pl:pl + 64, w0:w0 + O]
                acc[pl:pl + 64, m["slot0"] * O:(m["slot0"] + 1) * O] += \
                    lhsT.T @ rhs
        for s, p in enumerate(pairs):
            out[:, p, :] = acc[:, s * O:(s + 1) * O]
    return out  # [g*64+b, pair, o]; scale folded via x/32 * w*32


def _assemble(outs):
    """list of 8 per-core [128, NPAIR*O] (flat order) -> [B,O,ROWS,COLS] f32."""
    percore = []
    for o in outs:
        a = np.asarray(o, np.float32).reshape(2, B, NPAIR, O)
        percore.append(a.transpose(2, 0, 1, 3).reshape(PPC, B, O))
    full = np.concatenate(percore, axis=0)           # [784, B, O]
    return np.ascontiguousarray(full.transpose(1, 2, 0)).reshape(B, O, ROWS, COLS)


def _segments():
    """Cut each stripe's weight columns at matmul boundaries into ~SEG-byte
    DMA segments -> list of (stripe_idx, lo, hi) in stream order (absolute
    columns).  Stripe 0 gets a small leading segment for a fast start."""
    bases, tot = _stripe_bases()
    segs = []
    for si, pairs in enumerate(_stripes()):
        di, cs, mms, ncols = _stripe_plan(pairs)
        bounds = sorted({m["woff"] for m in mms} | {ncols})
        targets = [2048, 6656] if si == 0 else [SEG]
        cuts = [0]
        for tgt in targets:
            nxt = min((b for b in bounds if b >= tgt), default=ncols)
            if nxt < ncols and nxt > cuts[-1]:
                cuts.append(nxt)
        cuts.append(ncols)
        for lo, hi in zip(cuts, cuts[1:]):
            segs.append((si, bases[si] + lo, bases[si] + hi))
    return segs


_PROG_CACHE = {}


def _build_program():
    if "nc" in _PROG_CACHE:
        return _PROG_CACHE["nc"]
    import concourse.bass as bass
    import concourse.tile as tile
    from concourse import bacc, mybir

    f8, f16, f32 = mybir.dt.float8e3, mybir.dt.float16, mybir.dt.float32
    NOSYNC = mybir.DependencyInfo.NO_SYNC_ONLY
    bases, WTOT = _stripe_bases()
    segs = _segments()
    nc = bacc.Bacc("TRN2", target_bir_lowering=False, debug=False, num_devices=NCORES)
    xp_d = nc.dram_tensor("xp", [XROWS, 128, PAIRS * B], f16, kind="ExternalInput")
    wt_d = nc.dram_tensor("wt", [128, WTOT], f8, kind="ExternalInput")
    out_d = nc.dram_tensor("out", [128, NPAIR * O], f16, kind="ExternalOutput")

    with tile.TileContext(nc) as tc:
        with tc.tile_pool(name="xpool", bufs=1) as xpool, \
             tc.tile_pool(name="wpool", bufs=WLA) as wpool, \
             tc.tile_pool(name="opool", bufs=3) as opool, \
             tc.tile_pool(name="psum", bufs=6, space="PSUM") as ppool:
            xp, wt, outp = xp_d.ap(), wt_d.ap(), out_d.ap()
            XT = [xpool.tile([128, PAIRS * B], f16, name=f"x{h}", tag=f"x{h}")
                  for h in range(XROWS)]
            wtiles = [wpool.tile([128, hi - lo], f8, name=f"w{i}", tag="wt")
                      for i, (si, lo, hi) in enumerate(segs)]
            # map absolute weight column -> (segment idx, local offset)
            weng = [nc.scalar, nc.sync]

            def seg_of(col):
                for i, (si, lo, hi) in enumerate(segs):
                    if lo <= col < hi:
                        return i, col - lo
                raise AssertionError(col)

            def load_x(h, hf, eng):
                eng.dma_start(XT[h][:, hf * XHALF:(hf + 1) * XHALF],
                              xp[h, :, hf * XHALF:(hf + 1) * XHALF])

            def load_w(i):
                si, lo, hi = segs[i]
                weng[i % 2].dma_start(wtiles[i][:], wt[:, lo:hi])

            # Emission order == per-queue FIFO order.  Stripe 0 (di=0) needs
            # x rows 0-4 (first halves) + its leading weight segment.
            load_w(0)   # sync? weng[0]=scalar... keep explicit alternation below
            for h in range(3):
                load_x(h, 0, nc.sync)
            load_w(1)
            load_x(3, 0, nc.sync)
            load_x(4, 0, nc.sync)
            load_w(2)
            load_w(3)
            for h in range(5, XROWS):
                load_x(h, 0, nc.sync)
            load_w(4)
            for h in range(0, XROWS, 2):
                load_x(h, 1, nc.scalar)
            for h in range(1, XROWS, 2):
                load_x(h, 1, nc.sync)
            for i in range(5, len(segs)):
                load_w(i)  # flow-controlled by wpool depth

            # Tensor-queue program order is load-bearing (whole-bank clear
            # on each half's first matmul must precede the rest): chain all
            # tensor instructions with nosync deps.
            tprev = [None]

            def mm(out_ap, lhsT, rhs, start, stop):
                bi = nc.tensor.matmul(out_ap, lhsT, rhs, start=start, stop=stop)
                if tprev[0] is not None:
                    bi.ins.add_dependency(tprev[0], NOSYNC)
                tprev[0] = bi.ins.name
                return bi

            for si, pairs in enumerate(_stripes()):
                di, cs, mms, ncols = _stripe_plan(pairs)
                base = bases[si]
                n = len(pairs)
                stile = ppool.tile([128, n * O], f32, name="ps", tag="ps")
                otile = opool.tile([128, n * O], f16, name="ot", tag="ot")
                seen = [False, False]
                nlone = [0, 0]
                for m in mms:
                    h = di + m["u"]
                    gi, loc = seg_of(base + m["woff"])
                    wti = wtiles[gi]
                    pl = m["half"] * 64
                    start = not seen[m["half"]]
                    seen[m["half"]] = True
                    if m["kind"] == "pair":
                        mm(stile[pl:pl + 64,
                                 m["slot0"] * O:(m["slot0"] + m["nblk"]) * O],
                           XT[h][:, m["cp"] * B:(m["cp"] + 1) * B],
                           wti[:, loc:loc + m["nblk"] * O],
                           start, False)
                    else:
                        nlone[m["half"]] += 1
                        stop = nlone[m["half"]] == KH * n
                        mm(stile[pl:pl + 64,
                                 m["slot0"] * O:(m["slot0"] + 1) * O],
                           XT[h][pl:pl + 64, m["cp"] * B:(m["cp"] + 1) * B],
                           wti[pl:pl + 64, loc:loc + O],
                           start, stop)
                nc.vector.tensor_copy(otile[:], stile[:])
                oeng = nc.sync if si == len(_stripes()) - 1 else nc.gpsimd
                oeng.dma_start(
                    outp[:, pairs[0] * O:(pairs[0] + n) * O], otile[:])

    nc.compile()
    _PROG_CACHE["nc"] = nc
    return nc


def _make_in_maps(x, weight):
    x_chwb = np.ascontiguousarray(
        np.asarray(x, np.float32).transpose(1, 2, 3, 0))
    w32 = np.asarray(weight, np.float32)
    return [{"xp": _build_xp(x_chwb, k), "wt": _build_wt(w32, k)}
            for k in range(NCORES)]


def kernel(x, weight):
    from concourse.bass_utils import run_bass_kernel_spmd

    nc = _build_program()
    in_maps = _make_in_maps(x, weight)
    res = run_bass_kernel_spmd(nc, in_maps, core_ids=list(range(NCORES)))
    return _assemble([res.results[k]["out"] for k in range(NCORES)])


# revision 10
# speedup vs baseline: 2.4504x; 1.0341x over previous
"""Locally-connected convolution (unshared weights) on 8 Trainium2 NeuronCores.

out[b,o,i,j] = sum_{c,u,v} x[b,c,i+u,j+v] * weight[i,j,o,c,u,v]
  x: [64, 64, 32, 32] f32, weight: [28, 28, 128, 64, 5, 5] f32 -> out [64, 128, 28, 28]

"x-chunk-major" flipped layout: the weight stream (20.07 MB/core fp8, used
exactly once) is the hard floor (~60 us at ~350 GB/s HBM->SBUF), so the PE
must ingest weights faster than DMA delivers them.  Weights are the MOVING
matmul operand (2 concurrent col-tiled streams = 2 cols/cycle @ 2.4 GHz);
x chunks are the stationary operand, and the weight stream is reordered so
one stationary load serves up to 256 moving columns:

  For a stripe of 4 (even, odd) position pairs sharing an output row, the
  PSUM accumulator is one bank [128, 4*O] f32: partitions 0-63 = batch of
  the 4 even positions, 64-127 = odd; free = (pair slot, o).  An x chunk
  [128, 64] at (row h, colpair cp) is consumed by two adjacent even
  positions' weight blocks -> ONE matmul, rhs [128, 256], out two adjacent
  PSUM slots.  LDWEIGHTS (64 cols) hides under the 256-col stream.  Even
  and odd positions ride the two column halves of the PE array
  (tile_position (0,0) / (0,64)) so their streams run concurrently.
  Accumulation relies on per-element has_written: only the first matmul
  per (stripe, half) uses start=True (whole-bank clear); later matmuls
  overwrite-on-first-touch / accumulate, which is order-independent.
  The 5 leftover K=64 lone taps per position run as 64x64 quadrant
  matmuls ((0,0) vs (64,64)), two parities concurrent, sharing [128, O]
  weight blocks.  A nosync dependency chain pins tensor-queue program
  order (PE-array state + bank clears are resources the Tile dependency
  tracker doesn't model).

K decomposition per position (1600 = 25 taps x 64 ch) and the x layout are
unchanged from the pairs kernel: x stored once per core, partitions 0-63 =
channels of EVEN input columns, 64-127 ODD; 10 pair chunks + 5 lone taps.

Schedule: weight DMAs carry ~6400 B/partition segments cut at matmul
boundaries, alternating the two HWDGE queues (16 SDMA engines round-robin:
both queues busy => ~350 GB/s); a deep tile pool keeps DMA independent of
PE progress.  One [128, 4*O] f32->f16 cast per stripe; outputs stream per
stripe via gpsimd.  Output layout: [g*64+b, pair, o] for position 2*pair+g.
"""

import numpy as np

B, C, H, W = 64, 64, 32, 32
ROWS = COLS = 28
O, KH, KW = 128, 5, 5
NCORES = 8
PPC = (ROWS * COLS) // NCORES  # 98 positions per core
NPAIR = PPC // 2               # 49 (even, odd) position pairs
XROWS, XW = 8, 36              # sheared x grid: 8 input rows x 36 cols
PAIRS = XW // 2                # 18 column pairs per sheared row
WSCALE = 32.0                  # weights x32 into E3M4 range; x carries /32
SP = 4                         # pairs per stripe (= one PSUM bank)
WLA = 10                       # wtile pool depth (whole stripes in flight)
XHALF = 9 * B                  # x row loaded in two 9-pair halves


def _core_geom(k):
    p0 = PPC * k
    return p0 // COLS, p0 % COLS  # r0 (first input/output row), s0 in {0, 14}


def _pos_slot(t):
    """Relative position t in [0,98) -> (di, w2) grid coords shared by all cores."""
    di, jj = t // COLS, t % COLS
    return di, jj + (4 if jj >= 14 else 0)


def _pair_geom(p):
    """Pair p -> (di, c): positions 2p (w2=2c) and 2p+1 (w2=2c+1), same di."""
    di, w2 = _pos_slot(2 * p)
    return di, w2 // 2


def _stripes():
    """Stripes never straddle output rows (rows are 14 or 7 pairs)."""
    out = []
    for r0 in range(0, NPAIR, 14):
        row = list(range(r0, min(r0 + 14, NPAIR)))
        sizes = [4, 4, 3, 3] if len(row) == 14 else [4, 3]
        i = 0
        for sz in sizes:
            out.append(row[i:i + sz])
            i += sz
    return out


def _stripe_plan(pairs):
    """-> (di, cs, mms, ncols).  mms in emission order; weight columns are
    assigned in the same order (the A/B lone matmuls share one block).

    mm: dict(kind, half, u, cp, slot0, nblk, woff, blocks)
      pair: lhsT = XT[di+u][:, cp*B:(cp+1)*B], rhs = wt[:, woff:woff+nblk*O],
            out = stile[half*64:+64, slot0*O:(slot0+nblk)*O]
      lone: all three operands on partitions [half*64, half*64+64)
    """
    di, _ = _pair_geom(pairs[0])
    cs = [_pair_geom(p)[1] for p in pairs]
    n = len(pairs)
    mms = []
    off = 0
    for u in range(KH):
        per_half = []
        for half in range(2):
            blocks = [(s, q) for s in range(n) for q in range(2)]
            groups, i = [], 0
            while i < len(blocks):
                s, q = blocks[i]
                if (q == 1 and i + 1 < len(blocks)
                        and cs[blocks[i + 1][0]] == cs[s] + 1):
                    groups.append([blocks[i], blocks[i + 1]])
                    i += 2
                else:
                    groups.append([blocks[i]])
                    i += 1
            per_half.append(groups)
        ga, gb = per_half
        for j in range(max(len(ga), len(gb))):
            for half, gl in ((0, ga), (1, gb)):
                if j < len(gl):
                    grp = gl[j]
                    s0, q0 = grp[0]
                    mms.append(dict(kind="pair", half=half, u=u,
                                    cp=cs[s0] + q0 + half, slot0=s0,
                                    nblk=len(grp), woff=off, blocks=grp))
                    off += len(grp) * O
    for u in range(KH):
        for s in range(n):
            for half in range(2):
                mms.append(dict(kind="lone", half=half, u=u,
                                cp=cs[s] + (2 if half == 0 else 0),
                                slot0=s, nblk=1, woff=off, blocks=[(s, None)]))
            off += O  # one [128, O] block shared by the A and B lone matmuls
    return di, cs, mms, off


def _stripe_bases():
    bases, tot = [], 0
    for pairs in _stripes():
        bases.append(tot)
        tot += _stripe_plan(pairs)[3]
    return bases, tot


def _build_xs(x_chwb, k):
    """x_chwb: [C,H,W,B] f32 -> sheared per-core grid [C, XROWS, XW, B]."""
    r0, s0 = _core_geom(k)
    xs = np.zeros((C, XROWS, XW, B), dtype=x_chwb.dtype)
    for h in range(XROWS):
        if s0 == 0:
            xs[:, h, 0:18] = x_chwb[:, r0 + h, 0:18]
            xs[:, h, 18:36] = x_chwb[:, r0 + h, 14:32]
        else:
            xs[:, h, 0:18] = x_chwb[:, r0 + h, 14:32]
            if r0 + h + 1 < H:
                xs[:, h, 18:36] = x_chwb[:, r0 + h + 1, 0:18]
    return xs


def _build_xp(x_chwb, k):
    """-> [XROWS, 128, PAIRS*B] f16, partition g*64+c = channel c of col 2cp+g."""
    xs = _build_xs(x_chwb, k) * np.float32(1.0 / WSCALE)
    xg = xs.reshape(C, XROWS, PAIRS, 2, B).transpose(3, 0, 1, 2, 4)
    xp = xg.reshape(128, XROWS, PAIRS * B).transpose(1, 0, 2)
    return np.ascontiguousarray(xp).astype(np.float16)


def _abs_pos(k, t):
    p = PPC * k + t
    return p // COLS, p % COLS


def _build_wt(weight, k):
    """weight [ROWS,COLS,O,C,KH,KW] f32 -> per-core [128, WTOT] E3M4 in
    x-chunk-major stream order (exactly the _stripe_plan emission order)."""
    import ml_dtypes

    ii, jj = zip(*[_abs_pos(k, t) for t in range(PPC)])
    wc = weight[list(ii), list(jj)]  # [PPC, O, C, KH, KW] f32
    bases, tot = _stripe_bases()
    WT = np.zeros((128, tot), np.float32)
    for si, pairs in enumerate(_stripes()):
        di, cs, mms, ncols = _stripe_plan(pairs)
        base = bases[si]
        for m in mms:
            u = m["u"]
            if m["kind"] == "pair":
                t0 = m["half"]
                for bi, (s, q) in enumerate(m["blocks"]):
                    t = 2 * pairs[s] + t0
                    col = base + m["woff"] + bi * O
                    for g in range(2):
                        v = 2 * q + g + t0
                        # block[g*64+cch, o] = w[t][o, cch, u, v]
                        WT[g * 64:(g + 1) * 64, col:col + O] = \
                            wc[t, :, :, u, v].T
            elif m["half"] == 0:  # fill the shared lone block once
                s = m["slot0"]
                col = base + m["woff"]
                WT[0:64, col:col + O] = wc[2 * pairs[s], :, :, u, 4].T
                WT[64:128, col:col + O] = wc[2 * pairs[s] + 1, :, :, u, 0].T
    wt = np.ascontiguousarray(WT * np.float32(WSCALE))
    return wt.astype(ml_dtypes.float8_e3m4)


def _emulate_core(xp, wt):
    """Pure-numpy emulation of the device program (mirrors AP arithmetic)."""
    xpf = xp.astype(np.float32)
    wtf = wt.astype(np.float32)
    bases, _ = _stripe_bases()
    out = np.zeros((128, NPAIR, O), np.float32)
    for si, pairs in enumerate(_stripes()):
        di, cs, mms, ncols = _stripe_plan(pairs)
        base = bases[si]
        acc = np.zeros((128, len(pairs) * O), np.float32)
        for m in mms:
            h = di + m["u"]
            w0 = base + m["woff"]
            pl = m["half"] * 64
            if m["kind"] == "pair":
                lhsT = xpf[h][:, m["cp"] * B:(m["cp"] + 1) * B]
                rhs = wtf[:, w0:w0 + m["nblk"] * O]
                acc[pl:pl + 64, m["slot0"] * O:(m["slot0"] + m["nblk"]) * O] += \
                    lhsT.T @ rhs
            else:
                lhsT = xpf[h][pl:pl + 64, m["cp"] * B:(m["cp"] + 1) * B]
                rhs = wtf[pl:pl + 64, w0:w0 + O]
                acc[pl:pl + 64, m["slot0"] * O:(m["slot0"] + 1) * O] += \
                    lhsT.T @ rhs
        for s, p in enumerate(pairs):
            out[:, p, :] = acc[:, s * O:(s + 1) * O]
    return out  # [g*64+b, pair, o]; scale folded via x/32 * w*32


def _assemble(outs):
    """list of 8 per-core [128, NPAIR*O] (flat order) -> [B,O,ROWS,COLS] f32."""
    percore = []
    for o in outs:
        a = np.asarray(o, np.float32).reshape(2, B, NPAIR, O)
        percore.append(a.transpose(2, 0, 1, 3).reshape(PPC, B, O))
    full = np.concatenate(percore, axis=0)           # [784, B, O]
    return np.ascontiguousarray(full.transpose(1, 2, 0)).reshape(B, O, ROWS, COLS)


def _segments():
    """One whole-stripe DMA segment per stripe (1.2-1.6 MB transfers keep
    the SDMA engines at line rate with minimal per-transfer overhead);
    stripe 0 is cut at matmul boundaries into three small leading segments
    for a fast pipeline start.  -> list of (stripe_idx, lo, hi) absolute
    weight columns in stream order."""
    bases, tot = _stripe_bases()
    segs = []
    for si, pairs in enumerate(_stripes()):
        di, cs, mms, ncols = _stripe_plan(pairs)
        if si == 0:
            bounds = sorted({m["woff"] for m in mms} | {ncols})
            cuts = [0]
            for tgt in (2048, 6656):
                nxt = min((b for b in bounds if b >= tgt), default=ncols)
                if cuts[-1] < nxt < ncols:
                    cuts.append(nxt)
            cuts.append(ncols)
        else:
            cuts = [0, ncols]
        for lo, hi in zip(cuts, cuts[1:]):
            segs.append((si, bases[si] + lo, bases[si] + hi))
    return segs


_PROG_CACHE = {}


def _build_program():
    if "nc" in _PROG_CACHE:
        return _PROG_CACHE["nc"]
    import concourse.bass as bass
    import concourse.tile as tile
    from concourse import bacc, mybir

    f8, f16, f32 = mybir.dt.float8e3, mybir.dt.float16, mybir.dt.float32
    NOSYNC = mybir.DependencyInfo.NO_SYNC_ONLY
    bases, WTOT = _stripe_bases()
    segs = _segments()
    nc = bacc.Bacc("TRN2", target_bir_lowering=False, debug=False, num_devices=NCORES)
    xp_d = nc.dram_tensor("xp", [XROWS, 128, PAIRS * B], f16, kind="ExternalInput")
    wt_d = nc.dram_tensor("wt", [128, WTOT], f8, kind="ExternalInput")
    out_d = nc.dram_tensor("out", [128, NPAIR * O], f16, kind="ExternalOutput")

    with tile.TileContext(nc) as tc:
        with tc.tile_pool(name="xpool", bufs=1) as xpool, \
             tc.tile_pool(name="wpool", bufs=WLA) as wpool, \
             tc.tile_pool(name="opool", bufs=3) as opool, \
             tc.tile_pool(name="psum", bufs=6, space="PSUM") as ppool:
            xp, wt, outp = xp_d.ap(), wt_d.ap(), out_d.ap()
            XT = [xpool.tile([128, PAIRS * B], f16, name=f"x{h}", tag=f"x{h}")
                  for h in range(XROWS)]
            wtiles = [wpool.tile([128, hi - lo], f8, name=f"w{i}", tag="wt")
                      for i, (si, lo, hi) in enumerate(segs)]
            # map absolute weight column -> (segment idx, local offset)
            weng = [nc.scalar, nc.sync]
            qbytes = [0, 0]  # per-queue bytes/partition, greedily balanced

            def seg_of(col):
                for i, (si, lo, hi) in enumerate(segs):
                    if lo <= col < hi:
                        return i, col - lo
                raise AssertionError(col)

            def q_pick(nbytes):
                qi = 0 if qbytes[0] <= qbytes[1] else 1
                qbytes[qi] += nbytes
                return weng[qi]

            def load_x(h, hf):
                q_pick(2 * XHALF).dma_start(
                    XT[h][:, hf * XHALF:(hf + 1) * XHALF],
                    xp[h, :, hf * XHALF:(hf + 1) * XHALF])

            def load_w(i):
                si, lo, hi = segs[i]
                q_pick(hi - lo).dma_start(wtiles[i][:], wt[:, lo:hi])

            # Emission order == per-queue FIFO order.  Stripe 0 (di=0) needs
            # x rows 0-4 first halves + its leading weight segment; stripe 1
            # already reaches the second halves (cp up to 11), so those come
            # right after stripe 0's weights.
            load_w(0)
            for h in range(5):
                load_x(h, 0)
            load_w(1)
            load_w(2)
            for h in range(5):
                load_x(h, 1)
            load_w(3)
            for h in range(5, XROWS):
                load_x(h, 0)
            load_w(4)
            for h in range(5, XROWS):
                load_x(h, 1)
            for i in range(5, len(segs)):
                load_w(i)  # flow-controlled by wpool depth

            # Tensor-queue program order is load-bearing (whole-bank clear
            # on each half's first matmul must precede the rest): chain all
            # tensor instructions with nosync deps.
            tprev = [None]

            def mm(out_ap, lhsT, rhs, start, stop):
                bi = nc.tensor.matmul(out_ap, lhsT, rhs, start=start, stop=stop)
                if tprev[0] is not None:
                    bi.ins.add_dependency(tprev[0], NOSYNC)
                tprev[0] = bi.ins.name
                return bi

            for si, pairs in enumerate(_stripes()):
                di, cs, mms, ncols = _stripe_plan(pairs)
                base = bases[si]
                n = len(pairs)
                stile = ppool.tile([128, n * O], f32, name="ps", tag="ps")
                otile = opool.tile([128, n * O], f16, name="ot", tag="ot")
                seen = [False, False]
                nlone = [0, 0]
                for m in mms:
                    h = di + m["u"]
                    gi, loc = seg_of(base + m["woff"])
                    wti = wtiles[gi]
                    pl = m["half"] * 64
                    start = not seen[m["half"]]
                    seen[m["half"]] = True
                    if m["kind"] == "pair":
                        mm(stile[pl:pl + 64,
                                 m["slot0"] * O:(m["slot0"] + m["nblk"]) * O],
                           XT[h][:, m["cp"] * B:(m["cp"] + 1) * B],
                           wti[:, loc:loc + m["nblk"] * O],
                           start, False)
                    else:
                        nlone[m["half"]] += 1
                        stop = nlone[m["half"]] == KH * n
                        mm(stile[pl:pl + 64,
                                 m["slot0"] * O:(m["slot0"] + 1) * O],
                           XT[h][pl:pl + 64, m["cp"] * B:(m["cp"] + 1) * B],
                           wti[pl:pl + 64, loc:loc + O],
                           start, stop)
                nc.vector.tensor_copy(otile[:], stile[:])
                oeng = nc.sync if si == len(_stripes()) - 1 else nc.gpsimd
                oeng.dma_start(
                    outp[:, pairs[0] * O:(pairs[0] + n) * O], otile[:])

    nc.compile()
    _PROG_CACHE["nc"] = nc
    return nc


def _make_in_maps(x, weight):
    x_chwb = np.ascontiguousarray(
        np.asarray(x, np.float32).transpose(1, 2, 3, 0))
    w32 = np.asarray(weight, np.float32)
    return [{"xp": _build_xp(x_chwb, k), "wt": _build_wt(w32, k)}
            for k in range(NCORES)]


def kernel(x, weight):
    from concourse.bass_utils import run_bass_kernel_spmd

    nc = _build_program()
    in_maps = _make_in_maps(x, weight)
    res = run_bass_kernel_spmd(nc, in_maps, core_ids=list(range(NCORES)))
    return _assemble([res.results[k]["out"] for k in range(NCORES)])
